# revision 1
# baseline (speedup 1.0000x reference)
"""MoE layer (routed top-2 of 8 experts) on 8 TRN2 NeuronCores.

Contract: kernel(**inputs) takes the FULL unsharded inputs and returns the
FULL [4, 4096, 512] float32 output. Sharding/compile/run happens inside.

Primary design (expert-parallel routed, MOE_DESIGN=routed, default):
  - Host computes the gating (128x128 @ 128x8 matmul, top-2, softmax) and
    per-expert token index lists.
  - Core e owns expert e: it gathers its expert's tokens directly from the
    full fp16 x in DRAM with a transposed dma_gather (tokens land as
    [din, token] tiles, exactly the matmul stationary layout), runs the
    512x512 expert matmul in fp16 with fp32 PSUM accumulation, adds the
    bias (DVE) and applies tanh (ACT, casting to fp16), and streams the
    dense gathered result yg = tanh(x[idx] @ We + be) back to DRAM.
  - Host applies the gate weights and scatter-adds each core's yg rows into
    the final fp32 output (each core's row set is duplicate-free, so this
    is a plain fancy-index add).

Fallback (MOE_DESIGN=dense): data-parallel dense-expert kernel in float32r
(~4x slower, ~2e-4 more accurate).
"""

import os

import numpy as np

# The axon NTFF profiling hooks (antenv.axon_hooks) are not shipped in this
# container; BASS_TRACE=1 in the environment would crash run_bass_kernel_spmd.
os.environ["BASS_NEVER_TRACE"] = "1"

import concourse.bass as bass
import concourse.bacc as bacc
import concourse.mybir as mybir
from concourse.tile import TileContext
from concourse.bass_utils import run_bass_kernel_spmd

F32 = mybir.dt.float32
F32R = mybir.dt.float32r
BF16 = mybir.dt.bfloat16
I16 = mybir.dt.int16
F16 = mybir.dt.float16

NB, NLOC, DIN, DOUT = 4, 4096, 512, 512
NTYPES, TEBD = 128, 128
NE, TOPK = 8, 2
NCORES = 8
T = NB * NLOC          # 16384 tokens
TC = T // NCORES       # 2048 tokens per core
MT = TC // 128         # 16 token m-tiles per core
KT = DIN // 128        # 4 k-tiles

_cache = {}

# set by run (module-level so test.py can read timing/trace results)
last_results = None


def _routing(type_embeddings, Wg, atom_types):
    """Host-side gating math (tiny): per-token dense expert weights [T, NE]."""
    logits = type_embeddings.astype(np.float32) @ Wg.astype(np.float32)  # [NTYPES, NE]
    order = np.argsort(-logits, axis=-1, kind="stable")                  # stable => ties to lower idx
    top2 = order[:, :TOPK]                                               # [NTYPES, 2]
    tv = np.take_along_axis(logits, top2, axis=-1)                       # [NTYPES, 2]
    ex = np.exp(tv - tv.max(axis=-1, keepdims=True))
    w = (ex / ex.sum(axis=-1, keepdims=True)).astype(np.float32)         # [NTYPES, 2]
    ptw_types = np.zeros((NTYPES, NE), np.float32)
    np.put_along_axis(ptw_types, top2, w, axis=-1)
    at = atom_types.reshape(-1)
    return ptw_types[at], top2[at], w[at]  # dense [T, NE], idx [T,2], w [T,2]


def _build_dense():
    """Dense data-parallel program: out[t,:] = sum_e pw[t,e]*tanh(x[t]@We[e]+be[e])."""
    nc = bacc.Bacc("TRN2", target_bir_lowering=False, debug=False)
    # xt and We are packed into ONE param/DMA so matmuls have a single
    # producer semaphore (the fp32r LW matmul only fits one sync wait).
    xw_d = nc.declare_dram_parameter("xw", [128, KT * TC + NE * KT * DOUT], F32R, isOutput=False)
    pwl_d = nc.declare_dram_parameter("pwl", [128, MT * NE], F32, isOutput=False)
    ber_d = nc.declare_dram_parameter("ber", [128, NE * DOUT], F32, isOutput=False)
    out_d = nc.declare_dram_parameter("out", [TC, DOUT], F32, isOutput=True)

    with TileContext(nc) as tc:
        with (
            tc.tile_pool(name="const", bufs=1) as cpool,
            tc.tile_pool(name="work", bufs=4) as wpool,
            tc.tile_pool(name="psum", bufs=1, space="PSUM") as ppool,
        ):
            xw_sb = cpool.tile([128, KT * TC + NE * KT * DOUT], F32R)
            nc.sync.dma_start(xw_sb[:], xw_d[:])
            XOFF = KT * TC
            pwl_sb = cpool.tile([128, MT * NE], F32)
            nc.sync.dma_start(pwl_sb[:], pwl_d[:])
            ber_sb = cpool.tile([128, NE * DOUT], F32)
            nc.sync.dma_start(ber_sb[:], ber_d[:])

            for m in range(MT):
                pss = []
                for e in range(NE):
                    pss.append(
                        ppool.tile([128, DOUT], F32, name=f"ps{e}", tag=f"ps{e}")
                    )
                for k in range(KT):
                    lhs = xw_sb[:, k * TC + m * 128 : k * TC + (m + 1) * 128]
                    for e in range(NE):
                        nc.tensor.matmul(
                            pss[e][:],
                            lhs,
                            xw_sb[:, XOFF + (e * KT + k) * DOUT : XOFF + (e * KT + k + 1) * DOUT],
                            start=(k == 0),
                            stop=(k == KT - 1),
                        )
                acc = wpool.tile([128, DOUT], F32, tag="acc")
                for e in range(NE):
                    t1 = wpool.tile([128, DOUT], F32, tag="t1")
                    nc.vector.tensor_add(t1[:], pss[e][:], ber_sb[:, bass.ts(e, DOUT)])
                    t2 = wpool.tile([128, DOUT], F32, tag="t2")
                    nc.scalar.activation(
                        t2[:], t1[:], mybir.ActivationFunctionType.Tanh
                    )
                    wsc = pwl_sb[:, m * NE + e : m * NE + e + 1]
                    if e == 0:
                        nc.vector.tensor_scalar_mul(acc[:], t2[:], wsc)
                    else:
                        nc.vector.scalar_tensor_tensor(
                            acc[:],
                            t2[:],
                            wsc,
                            acc[:],
                            op0=mybir.AluOpType.mult,
                            op1=mybir.AluOpType.add,
                        )
                nc.sync.dma_start(out_d[bass.ts(m, 128), :], acc[:])
    nc.compile()
    return nc


GCHUNK = 256  # tokens per dma_gather (balances ~1us SWDGE fixed cost vs pipelining)


def _build_routed(cap):
    """Expert-parallel routed program (one expert per core).

    Each core gathers its expert's tokens from the full fp16 x in DRAM via
    transposed dma_gather ([din, token] tiles), runs the expert matmul in
    fp16 (fp32 accumulate), applies bias+tanh, and writes the dense gathered
    output yg = tanh(x@We+be) [cap, 512] fp16.  The per-token gate weight and
    the scatter-add into the final output happen on host (row sets are unique
    per core, so it is a plain fancy-index add).
    """
    mte = cap // 128
    nc = bacc.Bacc("TRN2", target_bir_lowering=False, debug=False)
    xb_d = nc.declare_dram_parameter("xb", [T, DIN], F16, isOutput=False)
    we_d = nc.declare_dram_parameter("we", [128, KT * DOUT], F16, isOutput=False)
    ber_d = nc.declare_dram_parameter("ber", [128, DOUT], F32, isOutput=False)
    gidx_d = nc.declare_dram_parameter("gidx", [128, cap // 16], I16, isOutput=False)
    yg_d = nc.declare_dram_parameter("yg", [cap, DOUT], F16, isOutput=True)

    with TileContext(nc) as tc:
        with (
            tc.tile_pool(name="const", bufs=1) as cpool,
            tc.tile_pool(name="xg", bufs=4) as xgpool,
            tc.tile_pool(name="work", bufs=6) as wpool,
            tc.tile_pool(name="psum", bufs=1, space="PSUM") as ppool,
        ):
            # idx first: HWDGE DMAs drain FIFO per engine, and the gathers
            # (critical-path head) wait on the index table.
            idx_sb = cpool.tile([128, cap // 16], I16)
            nc.sync.dma_start(idx_sb[:], gidx_d[:])
            we_sb = cpool.tile([128, KT * DOUT], F16)
            nc.sync.dma_start(we_sb[:], we_d[:])
            ber_sb = cpool.tile([128, DOUT], F32)
            nc.sync.dma_start(ber_sb[:], ber_d[:])

            # Chunk pattern: two small 128-token warm-up gathers so the PE
            # starts ~2us sooner, then 256-token chunks (the measured HW
            # optimum), remainder last.
            chunks = [128, 128] if cap > 256 else [cap]
            rest = cap - sum(chunks)
            chunks += [GCHUNK] * (rest // GCHUNK)
            if rest % GCHUNK:
                chunks.append(rest % GCHUNK)
            g0 = 0
            m = 0
            for glen in chunks:
                xgm = xgpool.tile([128, KT, glen], F16, name="xgm", tag="xgm")
                nc.gpsimd.dma_gather(
                    out_ap=xgm[:],
                    in_ap=xb_d[:],
                    idxs_ap=idx_sb[:, g0 // 16 : (g0 + glen) // 16],
                    num_idxs=glen,
                    num_idxs_reg=glen,
                    elem_size=DIN,
                    transpose=True,
                )
                for off in range(0, glen, 128):
                    ps = ppool.tile(
                        [128, DOUT], F32, name=f"ps{m % 8}", tag=f"ps{m % 8}"
                    )
                    for k in range(KT):
                        nc.tensor.matmul(
                            ps[:],
                            xgm[:, k, off : off + 128],
                            we_sb[:, bass.ts(k, DOUT)],
                            start=(k == 0),
                            stop=(k == KT - 1),
                        )
                    t1 = wpool.tile([128, DOUT], F32, tag="t1")
                    nc.vector.tensor_add(t1[:], ps[:], ber_sb[:])
                    yg = wpool.tile([128, DOUT], F16, tag="yg")
                    nc.scalar.activation(
                        yg[:], t1[:], mybir.ActivationFunctionType.Tanh
                    )
                    nc.sync.dma_start(yg_d[bass.ts(m, 128), :], yg[:])
                    m += 1
                g0 += glen
    nc.compile()
    return nc


def _kernel_routed(x, type_embeddings, atom_types, Wg, We, be):
    global last_results
    x = np.asarray(x, np.float32)
    We = np.asarray(We, np.float32)
    be = np.asarray(be, np.float32)
    _, top2_t, w_t = _routing(
        np.asarray(type_embeddings, np.float32),
        np.asarray(Wg, np.float32),
        np.asarray(atom_types),
    )  # top2_t [T,2], w_t [T,2]

    x2 = x.reshape(T, DIN)
    xb = x2.astype(np.float16)

    # per-expert token lists (ascending token order)
    glist, gw = [], []
    for e in range(NE):
        sel1 = np.nonzero(top2_t[:, 0] == e)[0]
        sel2 = np.nonzero(top2_t[:, 1] == e)[0]
        toks = np.concatenate([sel1, sel2])
        ws = np.concatenate([w_t[sel1, 0], w_t[sel2, 1]])
        o = np.argsort(toks, kind="stable")
        glist.append(toks[o])
        gw.append(ws[o].astype(np.float32))
    counts = [len(g) for g in glist]
    cap = ((max(counts) + 127) // 128) * 128

    if ("routed", cap) not in _cache:
        _cache[("routed", cap)] = _build_routed(cap)
    nc = _cache[("routed", cap)]

    in_maps = []
    for e in range(NE):
        cnt = counts[e]
        gidx = np.zeros(cap, np.int16)
        gidx[:cnt] = glist[e]
        wvec = np.zeros(cap, np.float32)
        wvec[:cnt] = gw[e]
        # idx table: position i at [i % 16, i // 16], replicated to all 8
        # GPSIMD-core partition groups (HW reads per-core copies).
        idx16 = np.ascontiguousarray(
            np.tile(gidx.reshape(cap // 16, 16).T, (8, 1))
        ).astype(np.int16)
        we_c = np.ascontiguousarray(
            We[e].reshape(KT, 128, DOUT).transpose(1, 0, 2)
        ).reshape(128, KT * DOUT).astype(np.float16)
        ber = np.ascontiguousarray(
            np.broadcast_to(be[e].reshape(1, DOUT), (128, DOUT))
        )
        in_maps.append({"xb": xb, "we": we_c, "ber": ber, "gidx": idx16})

    res = run_bass_kernel_spmd(nc, in_maps, list(range(NCORES)))
    last_results = res

    out_full = np.zeros((T, DOUT), np.float32)
    for e in range(NE):
        cnt = counts[e]
        yg = np.asarray(res.results[e]["yg"][:cnt]).astype(np.float32)
        out_full[glist[e]] += gw[e][:cnt, None] * yg
    return out_full.reshape(NB, NLOC, DOUT)


def _build_routed2(tpc, nprim):
    """Load-balanced expert-parallel program.

    Every core computes `tpc` 128-token tiles: the first `nprim` use the
    core's resident primary-expert weights; the remaining `nov` tiles use
    per-tile weights (+bias) DMA'd from DRAM, letting overloaded experts
    spill whole tiles to under-loaded cores.  Same gather/epilogue as
    _build_routed.
    """
    nov = tpc - nprim
    cap = tpc * 128
    nc = bacc.Bacc("TRN2", target_bir_lowering=False, debug=False)
    xb_d = nc.declare_dram_parameter("xb", [T, DIN], F16, isOutput=False)
    we_d = nc.declare_dram_parameter("we", [128, KT * DOUT], F16, isOutput=False)
    ber_d = nc.declare_dram_parameter("ber", [128, DOUT], F32, isOutput=False)
    wem_d = nc.declare_dram_parameter(
        "wem", [128, nov * KT * DOUT], F16, isOutput=False
    )
    bem_d = nc.declare_dram_parameter("bem", [128, nov * DOUT], F16, isOutput=False)
    gidx_d = nc.declare_dram_parameter("gidx", [128, cap // 16], I16, isOutput=False)
    yg_d = nc.declare_dram_parameter("yg", [cap, DOUT], F16, isOutput=True)

    with TileContext(nc) as tc:
        with (
            tc.tile_pool(name="const", bufs=1) as cpool,
            tc.tile_pool(name="xg", bufs=4) as xgpool,
            tc.tile_pool(name="ow", bufs=3) as opool,
            tc.tile_pool(name="work", bufs=6) as wpool,
            tc.tile_pool(name="psum", bufs=1, space="PSUM") as ppool,
        ):
            idx_sb = cpool.tile([128, cap // 16], I16)
            nc.sync.dma_start(idx_sb[:], gidx_d[:])
            we_sb = cpool.tile([128, KT * DOUT], F16)
            nc.sync.dma_start(we_sb[:], we_d[:])
            ber_sb = cpool.tile([128, DOUT], F32)
            nc.sync.dma_start(ber_sb[:], ber_d[:])

            xgs = {}
            ow = {}
            for m in range(tpc):
                if m % (GCHUNK // 128) == 0:
                    g0 = m * 128
                    glen = min(GCHUNK, cap - g0)
                    xgm = xgpool.tile([128, KT, glen], F16, name="xgm", tag="xgm")
                    nc.gpsimd.dma_gather(
                        out_ap=xgm[:],
                        in_ap=xb_d[:],
                        idxs_ap=idx_sb[:, g0 // 16 : (g0 + glen) // 16],
                        num_idxs=glen,
                        num_idxs_reg=glen,
                        elem_size=DIN,
                        transpose=True,
                    )
                    xgs[m // (GCHUNK // 128)] = xgm
                xgm = xgs[m // (GCHUNK // 128)]
                off = (m % (GCHUNK // 128)) * 128
                if m < nprim:
                    wsrc, bsrc = we_sb, ber_sb[:]
                    woff = 0
                else:
                    # overflow tile: stream this slot's weights+bias from DRAM
                    # (emitted at use site so Tile schedules them just-in-time,
                    # prefetched `bufs` slots ahead, instead of up-front where
                    # they would starve the gathers of SDMA bandwidth)
                    j = m - nprim
                    wj = opool.tile([128, KT * DOUT], F16, name="wj", tag="wj")
                    nc.sync.dma_start(
                        wj[:], wem_d[:, j * KT * DOUT : (j + 1) * KT * DOUT]
                    )
                    bj = opool.tile([128, DOUT], F16, name="bj", tag="bj")
                    nc.sync.dma_start(bj[:], bem_d[:, j * DOUT : (j + 1) * DOUT])
                    wsrc, bsrc = wj, bj[:]
                    woff = 0
                ps = ppool.tile([128, DOUT], F32, name=f"ps{m % 8}", tag=f"ps{m % 8}")
                for k in range(KT):
                    nc.tensor.matmul(
                        ps[:],
                        xgm[:, k, off : off + 128],
                        wsrc[:, woff + k * DOUT : woff + (k + 1) * DOUT],
                        start=(k == 0),
                        stop=(k == KT - 1),
                    )
                t1 = wpool.tile([128, DOUT], F32, tag="t1")
                nc.vector.tensor_add(t1[:], ps[:], bsrc)
                yg = wpool.tile([128, DOUT], F16, tag="yg")
                nc.scalar.activation(yg[:], t1[:], mybir.ActivationFunctionType.Tanh)
                nc.sync.dma_start(yg_d[bass.ts(m, 128), :], yg[:])
    nc.compile()
    return nc


def _plan_balance(counts):
    """Pick (tpc, nprim) and assign each expert's 128-token tiles to cores.

    Returns (tpc, nprim, assign) where assign[c] is a list of length tpc of
    (expert, start, length) pieces ((c, 0, 0)-style dummies have length 0).
    Slot m < nprim must hold expert c (the core's resident expert); slots
    m >= nprim may hold any expert (weights come via the wem input).
    """
    ne = len(counts)
    ceils = [(c + 127) // 128 for c in counts]
    total = sum(ceils)
    tpc0 = max((total + ne - 1) // ne, 1)
    for tpc in range(tpc0, tpc0 + 64):
        nprim = None
        for cand in range(tpc, -1, -1):
            spill = sum(max(ce - cand, 0) for ce in ceils)
            if spill <= ne * (tpc - cand):
                nprim = cand
                break
        if nprim is not None:
            break
    assert nprim is not None
    # primary slots: expert c's first min(ceil_c, nprim) tiles on core c
    assign = []
    spill_tiles = []
    for e in range(ne):
        nown = min(ceils[e], nprim)
        tiles = [(e, t * 128, min(128, counts[e] - t * 128)) for t in range(ceils[e])]
        own = tiles[:nown] + [(e, 0, 0)] * (nprim - nown)
        assign.append(own)
        spill_tiles.extend(tiles[nown:])
    # overflow slots round-robin
    nov = tpc - nprim
    for c in range(ne):
        take, spill_tiles = spill_tiles[:nov], spill_tiles[nov:]
        take = take + [(c, 0, 0)] * (nov - take.__len__())
        assign[c] = assign[c] + take
    assert not spill_tiles
    return tpc, nprim, assign


def _kernel_routed2(x, type_embeddings, atom_types, Wg, We, be):
    global last_results
    x = np.asarray(x, np.float32)
    We = np.asarray(We, np.float32)
    be = np.asarray(be, np.float32)
    _, top2_t, w_t = _routing(
        np.asarray(type_embeddings, np.float32),
        np.asarray(Wg, np.float32),
        np.asarray(atom_types),
    )
    xb = x.reshape(T, DIN).astype(np.float16)

    glist, gw = [], []
    for e in range(NE):
        sel1 = np.nonzero(top2_t[:, 0] == e)[0]
        sel2 = np.nonzero(top2_t[:, 1] == e)[0]
        toks = np.concatenate([sel1, sel2])
        ws = np.concatenate([w_t[sel1, 0], w_t[sel2, 1]])
        o = np.argsort(toks, kind="stable")
        glist.append(toks[o])
        gw.append(ws[o].astype(np.float32))
    counts = [len(g) for g in glist]

    tpc, nprim, assign = _plan_balance(counts)
    nov = tpc - nprim
    cap = tpc * 128
    if ("routed2", tpc, nprim) not in _cache:
        _cache[("routed2", tpc, nprim)] = _build_routed2(tpc, nprim)
    nc = _cache[("routed2", tpc, nprim)]

    we_h = [
        np.ascontiguousarray(We[e].reshape(KT, 128, DOUT).transpose(1, 0, 2))
        .reshape(128, KT * DOUT)
        .astype(np.float16)
        for e in range(NE)
    ]
    ber_h = [
        np.ascontiguousarray(np.broadcast_to(be[e].reshape(1, DOUT), (128, DOUT)))
        for e in range(NE)
    ]
    in_maps = []
    for c in range(NCORES):
        gidx = np.zeros(cap, np.int16)
        for m, (e, s, L) in enumerate(assign[c]):
            if L:
                gidx[m * 128 : m * 128 + L] = glist[e][s : s + L]
        idx16 = np.ascontiguousarray(
            np.tile(gidx.reshape(cap // 16, 16).T, (8, 1))
        ).astype(np.int16)
        wem = np.concatenate(
            [we_h[e] for (e, s, L) in assign[c][nprim:]], axis=1
        ) if nov else np.zeros((128, 0), np.float16)
        bem = np.concatenate(
            [ber_h[e].astype(np.float16) for (e, s, L) in assign[c][nprim:]], axis=1
        ) if nov else np.zeros((128, 0), np.float16)
        in_maps.append(
            {
                "xb": xb,
                "we": we_h[c],
                "ber": ber_h[c],
                "wem": np.ascontiguousarray(wem),
                "bem": np.ascontiguousarray(bem),
                "gidx": idx16,
            }
        )

    res = run_bass_kernel_spmd(nc, in_maps, list(range(NCORES)))
    last_results = res

    out_full = np.zeros((T, DOUT), np.float32)
    # accumulate per expert (each expert's tiles partition its token list,
    # so indices are unique within one fancy-index add)
    for e in range(NE):
        ids, rows, ws = [], [], []
        for c in range(NCORES):
            yg = None
            for m, (te, s, L) in enumerate(assign[c]):
                if te == e and L:
                    if yg is None:
                        yg = np.asarray(res.results[c]["yg"])
                    ids.append(glist[e][s : s + L])
                    rows.append(yg[m * 128 : m * 128 + L])
                    ws.append(gw[e][s : s + L])
        if ids:
            ids = np.concatenate(ids)
            rows = np.concatenate(rows).astype(np.float32)
            ws = np.concatenate(ws)
            out_full[ids] += ws[:, None] * rows
    return out_full.reshape(NB, NLOC, DOUT)


def kernel(x, type_embeddings, atom_types, Wg, We, be):
    global last_results
    design = os.environ.get("MOE_DESIGN", "routed")
    if design == "routed2":
        return _kernel_routed2(x, type_embeddings, atom_types, Wg, We, be)
    if design == "routed":
        return _kernel_routed(x, type_embeddings, atom_types, Wg, We, be)
    x = np.asarray(x, np.float32)
    We = np.asarray(We, np.float32)
    be = np.asarray(be, np.float32)
    ptw, _, _ = _routing(
        np.asarray(type_embeddings, np.float32),
        np.asarray(Wg, np.float32),
        np.asarray(atom_types),
    )

    x2 = x.reshape(T, DIN)
    ber = np.ascontiguousarray(
        np.broadcast_to(be.reshape(1, NE * DOUT), (128, NE * DOUT))
    )
    # [128, NE*KT*DOUT]: we_h[p, (e*KT+k)*DOUT + d] = We[e, k*128+p, d]
    we_h = np.ascontiguousarray(
        We.reshape(NE, KT, 128, DOUT).transpose(2, 0, 1, 3)
    ).reshape(128, NE * KT * DOUT)
    in_maps = []
    for c in range(NCORES):
        x2c = x2[c * TC : (c + 1) * TC]
        # [128, KT*TC]: xt[p, k*TC + n] = x2c[n, k*128+p]
        xt = np.ascontiguousarray(
            x2c.reshape(TC, KT, 128).transpose(2, 1, 0)
        ).reshape(128, KT * TC)
        xw = np.concatenate([xt, we_h], axis=1)
        pwl = np.ascontiguousarray(
            ptw[c * TC : (c + 1) * TC].reshape(MT, 128, NE).transpose(1, 0, 2)
        ).reshape(128, MT * NE)
        in_maps.append({"xw": xw, "pwl": pwl, "ber": ber})

    if "dense" not in _cache:
        _cache["dense"] = _build_dense()
    nc = _cache["dense"]

    res = run_bass_kernel_spmd(nc, in_maps, list(range(NCORES)))
    last_results = res
    out = np.concatenate([res.results[c]["out"] for c in range(NCORES)], axis=0)
    return out.reshape(NB, NLOC, DOUT).astype(np.float32)



# revision 39
# speedup vs baseline: 1.2338x; 1.2338x over previous
"""MoE layer (routed top-2 of 8 experts) on 8 TRN2 NeuronCores.

Contract: kernel(**inputs) takes the FULL unsharded inputs and returns the
FULL [4, 4096, 512] float32 output. Sharding/compile/run happens inside.

Primary design (expert-parallel routed, MOE_DESIGN=routed, default):
  - Host computes the gating (128x128 @ 128x8 matmul, top-2, softmax) and
    per-expert token index lists.
  - Core e owns expert e: it gathers its expert's tokens directly from the
    full fp16 x in DRAM with a transposed dma_gather (tokens land as
    [din, token] tiles, exactly the matmul stationary layout), runs the
    512x512 expert matmul in fp16 with fp32 PSUM accumulation, adds the
    bias (DVE) and applies tanh (ACT, casting to fp16), and streams the
    dense gathered result yg = tanh(x[idx] @ We + be) back to DRAM.
  - Host applies the gate weights and scatter-adds each core's yg rows into
    the final fp32 output (each core's row set is duplicate-free, so this
    is a plain fancy-index add).

Fallback (MOE_DESIGN=dense): data-parallel dense-expert kernel in float32r
(~4x slower, ~2e-4 more accurate).
"""

import os

import numpy as np

# The axon NTFF profiling hooks (antenv.axon_hooks) are not shipped in this
# container; BASS_TRACE=1 in the environment would crash run_bass_kernel_spmd.
os.environ["BASS_NEVER_TRACE"] = "1"

import concourse.bass as bass
import concourse.bacc as bacc
import concourse.mybir as mybir
from concourse.tile import TileContext
from concourse.bass_utils import run_bass_kernel_spmd

F32 = mybir.dt.float32
F32R = mybir.dt.float32r
BF16 = mybir.dt.bfloat16
I16 = mybir.dt.int16
F16 = mybir.dt.float16

NB, NLOC, DIN, DOUT = 4, 4096, 512, 512
NTYPES, TEBD = 128, 128
NE, TOPK = 8, 2
NCORES = 8
T = NB * NLOC          # 16384 tokens
TC = T // NCORES       # 2048 tokens per core
MT = TC // 128         # 16 token m-tiles per core
KT = DIN // 128        # 4 k-tiles

_cache = {}

# set by run (module-level so test.py can read timing/trace results)
last_results = None
last_nc = None  # compiled Bass program used by the last kernel() call


def _routing(type_embeddings, Wg, atom_types):
    """Host-side gating math (tiny): per-token dense expert weights [T, NE]."""
    logits = type_embeddings.astype(np.float32) @ Wg.astype(np.float32)  # [NTYPES, NE]
    order = np.argsort(-logits, axis=-1, kind="stable")                  # stable => ties to lower idx
    top2 = order[:, :TOPK]                                               # [NTYPES, 2]
    tv = np.take_along_axis(logits, top2, axis=-1)                       # [NTYPES, 2]
    ex = np.exp(tv - tv.max(axis=-1, keepdims=True))
    w = (ex / ex.sum(axis=-1, keepdims=True)).astype(np.float32)         # [NTYPES, 2]
    ptw_types = np.zeros((NTYPES, NE), np.float32)
    np.put_along_axis(ptw_types, top2, w, axis=-1)
    at = atom_types.reshape(-1)
    return ptw_types[at], top2[at], w[at]  # dense [T, NE], idx [T,2], w [T,2]


def _build_dense():
    """Dense data-parallel program: out[t,:] = sum_e pw[t,e]*tanh(x[t]@We[e]+be[e])."""
    nc = bacc.Bacc("TRN2", target_bir_lowering=False, debug=False)
    # xt and We are packed into ONE param/DMA so matmuls have a single
    # producer semaphore (the fp32r LW matmul only fits one sync wait).
    xw_d = nc.declare_dram_parameter("xw", [128, KT * TC + NE * KT * DOUT], F32R, isOutput=False)
    pwl_d = nc.declare_dram_parameter("pwl", [128, MT * NE], F32, isOutput=False)
    ber_d = nc.declare_dram_parameter("ber", [128, NE * DOUT], F32, isOutput=False)
    out_d = nc.declare_dram_parameter("out", [TC, DOUT], F32, isOutput=True)

    with TileContext(nc) as tc:
        with (
            tc.tile_pool(name="const", bufs=1) as cpool,
            tc.tile_pool(name="work", bufs=4) as wpool,
            tc.tile_pool(name="psum", bufs=1, space="PSUM") as ppool,
        ):
            xw_sb = cpool.tile([128, KT * TC + NE * KT * DOUT], F32R)
            nc.sync.dma_start(xw_sb[:], xw_d[:])
            XOFF = KT * TC
            pwl_sb = cpool.tile([128, MT * NE], F32)
            nc.sync.dma_start(pwl_sb[:], pwl_d[:])
            ber_sb = cpool.tile([128, NE * DOUT], F32)
            nc.sync.dma_start(ber_sb[:], ber_d[:])

            for m in range(MT):
                pss = []
                for e in range(NE):
                    pss.append(
                        ppool.tile([128, DOUT], F32, name=f"ps{e}", tag=f"ps{e}")
                    )
                for k in range(KT):
                    lhs = xw_sb[:, k * TC + m * 128 : k * TC + (m + 1) * 128]
                    for e in range(NE):
                        nc.tensor.matmul(
                            pss[e][:],
                            lhs,
                            xw_sb[:, XOFF + (e * KT + k) * DOUT : XOFF + (e * KT + k + 1) * DOUT],
                            start=(k == 0),
                            stop=(k == KT - 1),
                        )
                acc = wpool.tile([128, DOUT], F32, tag="acc")
                for e in range(NE):
                    t1 = wpool.tile([128, DOUT], F32, tag="t1")
                    nc.vector.tensor_add(t1[:], pss[e][:], ber_sb[:, bass.ts(e, DOUT)])
                    t2 = wpool.tile([128, DOUT], F32, tag="t2")
                    nc.scalar.activation(
                        t2[:], t1[:], mybir.ActivationFunctionType.Tanh
                    )
                    wsc = pwl_sb[:, m * NE + e : m * NE + e + 1]
                    if e == 0:
                        nc.vector.tensor_scalar_mul(acc[:], t2[:], wsc)
                    else:
                        nc.vector.scalar_tensor_tensor(
                            acc[:],
                            t2[:],
                            wsc,
                            acc[:],
                            op0=mybir.AluOpType.mult,
                            op1=mybir.AluOpType.add,
                        )
                nc.sync.dma_start(out_d[bass.ts(m, 128), :], acc[:])
    nc.compile()
    return nc


GCHUNK = 256  # tokens per dma_gather (balances ~1us SWDGE fixed cost vs pipelining)


def _build_routed(cap):
    """Expert-parallel routed program (one expert per core).

    Each core gathers its expert's tokens from the full fp16 x in DRAM via
    transposed dma_gather ([din, token] tiles), runs the expert matmul in
    fp16 (fp32 accumulate), applies bias+tanh, and writes the dense gathered
    output yg = tanh(x@We+be) [cap, 512] fp16.  The per-token gate weight and
    the scatter-add into the final output happen on host (row sets are unique
    per core, so it is a plain fancy-index add).
    """
    mte = cap // 128
    nc = bacc.Bacc("TRN2", target_bir_lowering=False, debug=False)
    xb_d = nc.declare_dram_parameter("xb", [T, DIN], F16, isOutput=False)
    we_d = nc.declare_dram_parameter("we", [128, KT * DOUT], F16, isOutput=False)
    ber_d = nc.declare_dram_parameter("ber", [128, DOUT], F32, isOutput=False)
    gidx_d = nc.declare_dram_parameter("gidx", [128, cap // 16], I16, isOutput=False)
    yg_d = nc.declare_dram_parameter("yg", [cap, DOUT], F16, isOutput=True)

    with TileContext(nc) as tc:
        with (
            tc.tile_pool(name="const", bufs=1) as cpool,
            tc.tile_pool(name="xg", bufs=4) as xgpool,
            tc.tile_pool(name="work", bufs=6) as wpool,
            tc.tile_pool(name="psum", bufs=1, space="PSUM") as ppool,
        ):
            # idx first: HWDGE DMAs drain FIFO per engine, and the gathers
            # (critical-path head) wait on the index table.
            idx_sb = cpool.tile([128, cap // 16], I16)
            nc.sync.dma_start(idx_sb[:], gidx_d[:])
            we_sb = cpool.tile([128, KT * DOUT], F16)
            nc.sync.dma_start(we_sb[:], we_d[:])
            ber_sb = cpool.tile([128, DOUT], F32)
            nc.sync.dma_start(ber_sb[:], ber_d[:])

            # Chunk pattern: two small 128-token warm-up gathers so the PE
            # starts ~2us sooner, then 256-token chunks (the measured HW
            # optimum), remainder last.
            chunks = [128, 128] if cap > 256 else [cap]
            rest = cap - sum(chunks)
            chunks += [GCHUNK] * (rest // GCHUNK)
            if rest % GCHUNK:
                chunks.append(rest % GCHUNK)
            g0 = 0
            m = 0
            for glen in chunks:
                xgm = xgpool.tile([128, KT, glen], F16, name="xgm", tag="xgm")
                nc.gpsimd.dma_gather(
                    out_ap=xgm[:],
                    in_ap=xb_d[:],
                    idxs_ap=idx_sb[:, g0 // 16 : (g0 + glen) // 16],
                    num_idxs=glen,
                    num_idxs_reg=glen,
                    elem_size=DIN,
                    transpose=True,
                )
                for off in range(0, glen, 128):
                    ps = ppool.tile(
                        [128, DOUT], F32, name=f"ps{m % 8}", tag=f"ps{m % 8}"
                    )
                    for k in range(KT):
                        nc.tensor.matmul(
                            ps[:],
                            xgm[:, k, off : off + 128],
                            we_sb[:, bass.ts(k, DOUT)],
                            start=(k == 0),
                            stop=(k == KT - 1),
                        )
                    t1 = wpool.tile([128, DOUT], F32, tag="t1")
                    nc.vector.tensor_add(t1[:], ps[:], ber_sb[:])
                    yg = wpool.tile([128, DOUT], F16, tag="yg")
                    nc.scalar.activation(
                        yg[:], t1[:], mybir.ActivationFunctionType.Tanh
                    )
                    nc.sync.dma_start(yg_d[bass.ts(m, 128), :], yg[:])
                    m += 1
                g0 += glen
    nc.compile()
    return nc


def _kernel_routed(x, type_embeddings, atom_types, Wg, We, be):
    global last_results
    x = np.asarray(x, np.float32)
    We = np.asarray(We, np.float32)
    be = np.asarray(be, np.float32)
    _, top2_t, w_t = _routing(
        np.asarray(type_embeddings, np.float32),
        np.asarray(Wg, np.float32),
        np.asarray(atom_types),
    )  # top2_t [T,2], w_t [T,2]

    x2 = x.reshape(T, DIN)
    xb = x2.astype(np.float16)

    # per-expert token lists (ascending token order)
    glist, gw = [], []
    for e in range(NE):
        sel1 = np.nonzero(top2_t[:, 0] == e)[0]
        sel2 = np.nonzero(top2_t[:, 1] == e)[0]
        toks = np.concatenate([sel1, sel2])
        ws = np.concatenate([w_t[sel1, 0], w_t[sel2, 1]])
        o = np.argsort(toks, kind="stable")
        glist.append(toks[o])
        gw.append(ws[o].astype(np.float32))
    counts = [len(g) for g in glist]
    cap = ((max(counts) + 127) // 128) * 128

    if ("routed", cap) not in _cache:
        _cache[("routed", cap)] = _build_routed(cap)
    nc = _cache[("routed", cap)]

    in_maps = []
    for e in range(NE):
        cnt = counts[e]
        gidx = np.zeros(cap, np.int16)
        gidx[:cnt] = glist[e]
        wvec = np.zeros(cap, np.float32)
        wvec[:cnt] = gw[e]
        # idx table: position i at [i % 16, i // 16], replicated to all 8
        # GPSIMD-core partition groups (HW reads per-core copies).
        idx16 = np.ascontiguousarray(
            np.tile(gidx.reshape(cap // 16, 16).T, (8, 1))
        ).astype(np.int16)
        we_c = np.ascontiguousarray(
            We[e].reshape(KT, 128, DOUT).transpose(1, 0, 2)
        ).reshape(128, KT * DOUT).astype(np.float16)
        ber = np.ascontiguousarray(
            np.broadcast_to(be[e].reshape(1, DOUT), (128, DOUT))
        )
        in_maps.append({"xb": xb, "we": we_c, "ber": ber, "gidx": idx16})

    res = run_bass_kernel_spmd(nc, in_maps, list(range(NCORES)))
    last_results = res

    out_full = np.zeros((T, DOUT), np.float32)
    for e in range(NE):
        cnt = counts[e]
        yg = np.asarray(res.results[e]["yg"][:cnt]).astype(np.float32)
        out_full[glist[e]] += gw[e][:cnt, None] * yg
    return out_full.reshape(NB, NLOC, DOUT)


def _build_routed2(tpc, nprim):
    """Load-balanced expert-parallel program.

    Every core computes `tpc` 128-token tiles: the first `nprim` use the
    core's resident primary-expert weights; the remaining `nov` tiles use
    per-tile weights (+bias) DMA'd from DRAM, letting overloaded experts
    spill whole tiles to under-loaded cores.  Same gather/epilogue as
    _build_routed.
    """
    nov = tpc - nprim
    cap = tpc * 128
    nc = bacc.Bacc("TRN2", target_bir_lowering=False, debug=False)
    xb_d = nc.declare_dram_parameter("xb", [T, DIN], F16, isOutput=False)
    we_d = nc.declare_dram_parameter("we", [128, KT * DOUT], F16, isOutput=False)
    ber_d = nc.declare_dram_parameter("ber", [128, DOUT], F32, isOutput=False)
    wem_d = nc.declare_dram_parameter(
        "wem", [128, nov * KT * DOUT], F16, isOutput=False
    )
    bem_d = nc.declare_dram_parameter("bem", [128, nov * DOUT], F16, isOutput=False)
    gidx_d = nc.declare_dram_parameter("gidx", [128, cap // 16], I16, isOutput=False)
    yg_d = nc.declare_dram_parameter("yg", [cap, DOUT], F16, isOutput=True)

    with TileContext(nc) as tc:
        with (
            tc.tile_pool(name="const", bufs=1) as cpool,
            tc.tile_pool(name="xg", bufs=4) as xgpool,
            tc.tile_pool(name="ow", bufs=3) as opool,
            tc.tile_pool(name="work", bufs=6) as wpool,
            tc.tile_pool(name="psum", bufs=1, space="PSUM") as ppool,
        ):
            idx_sb = cpool.tile([128, cap // 16], I16)
            nc.sync.dma_start(idx_sb[:], gidx_d[:])
            we_sb = cpool.tile([128, KT * DOUT], F16)
            nc.sync.dma_start(we_sb[:], we_d[:])
            ber_sb = cpool.tile([128, DOUT], F32)
            nc.sync.dma_start(ber_sb[:], ber_d[:])

            xgs = {}
            ow = {}
            for m in range(tpc):
                if m % (GCHUNK // 128) == 0:
                    g0 = m * 128
                    glen = min(GCHUNK, cap - g0)
                    xgm = xgpool.tile([128, KT, glen], F16, name="xgm", tag="xgm")
                    nc.gpsimd.dma_gather(
                        out_ap=xgm[:],
                        in_ap=xb_d[:],
                        idxs_ap=idx_sb[:, g0 // 16 : (g0 + glen) // 16],
                        num_idxs=glen,
                        num_idxs_reg=glen,
                        elem_size=DIN,
                        transpose=True,
                    )
                    xgs[m // (GCHUNK // 128)] = xgm
                xgm = xgs[m // (GCHUNK // 128)]
                off = (m % (GCHUNK // 128)) * 128
                if m < nprim:
                    wsrc, bsrc = we_sb, ber_sb[:]
                    woff = 0
                else:
                    # overflow tile: stream this slot's weights+bias from DRAM
                    # (emitted at use site so Tile schedules them just-in-time,
                    # prefetched `bufs` slots ahead, instead of up-front where
                    # they would starve the gathers of SDMA bandwidth)
                    j = m - nprim
                    wj = opool.tile([128, KT * DOUT], F16, name="wj", tag="wj")
                    nc.sync.dma_start(
                        wj[:], wem_d[:, j * KT * DOUT : (j + 1) * KT * DOUT]
                    )
                    bj = opool.tile([128, DOUT], F16, name="bj", tag="bj")
                    nc.sync.dma_start(bj[:], bem_d[:, j * DOUT : (j + 1) * DOUT])
                    wsrc, bsrc = wj, bj[:]
                    woff = 0
                ps = ppool.tile([128, DOUT], F32, name=f"ps{m % 8}", tag=f"ps{m % 8}")
                for k in range(KT):
                    nc.tensor.matmul(
                        ps[:],
                        xgm[:, k, off : off + 128],
                        wsrc[:, woff + k * DOUT : woff + (k + 1) * DOUT],
                        start=(k == 0),
                        stop=(k == KT - 1),
                    )
                t1 = wpool.tile([128, DOUT], F32, tag="t1")
                nc.vector.tensor_add(t1[:], ps[:], bsrc)
                yg = wpool.tile([128, DOUT], F16, tag="yg")
                nc.scalar.activation(yg[:], t1[:], mybir.ActivationFunctionType.Tanh)
                nc.sync.dma_start(yg_d[bass.ts(m, 128), :], yg[:])
    nc.compile()
    return nc


def _plan_balance(counts):
    """Pick (tpc, nprim) and assign each expert's 128-token tiles to cores.

    Returns (tpc, nprim, assign) where assign[c] is a list of length tpc of
    (expert, start, length) pieces ((c, 0, 0)-style dummies have length 0).
    Slot m < nprim must hold expert c (the core's resident expert); slots
    m >= nprim may hold any expert (weights come via the wem input).
    """
    ne = len(counts)
    ceils = [(c + 127) // 128 for c in counts]
    total = sum(ceils)
    tpc0 = max((total + ne - 1) // ne, 1)
    for tpc in range(tpc0, tpc0 + 64):
        nprim = None
        for cand in range(tpc, -1, -1):
            spill = sum(max(ce - cand, 0) for ce in ceils)
            if spill <= ne * (tpc - cand):
                nprim = cand
                break
        if nprim is not None:
            break
    assert nprim is not None
    # primary slots: expert c's first min(ceil_c, nprim) tiles on core c
    assign = []
    spill_tiles = []
    for e in range(ne):
        nown = min(ceils[e], nprim)
        tiles = [(e, t * 128, min(128, counts[e] - t * 128)) for t in range(ceils[e])]
        own = tiles[:nown] + [(e, 0, 0)] * (nprim - nown)
        assign.append(own)
        spill_tiles.extend(tiles[nown:])
    # overflow slots round-robin
    nov = tpc - nprim
    for c in range(ne):
        take, spill_tiles = spill_tiles[:nov], spill_tiles[nov:]
        take = take + [(c, 0, 0)] * (nov - take.__len__())
        assign[c] = assign[c] + take
    assert not spill_tiles
    return tpc, nprim, assign


def _kernel_routed2(x, type_embeddings, atom_types, Wg, We, be):
    global last_results
    x = np.asarray(x, np.float32)
    We = np.asarray(We, np.float32)
    be = np.asarray(be, np.float32)
    _, top2_t, w_t = _routing(
        np.asarray(type_embeddings, np.float32),
        np.asarray(Wg, np.float32),
        np.asarray(atom_types),
    )
    xb = x.reshape(T, DIN).astype(np.float16)

    glist, gw = [], []
    for e in range(NE):
        sel1 = np.nonzero(top2_t[:, 0] == e)[0]
        sel2 = np.nonzero(top2_t[:, 1] == e)[0]
        toks = np.concatenate([sel1, sel2])
        ws = np.concatenate([w_t[sel1, 0], w_t[sel2, 1]])
        o = np.argsort(toks, kind="stable")
        glist.append(toks[o])
        gw.append(ws[o].astype(np.float32))
    counts = [len(g) for g in glist]

    tpc, nprim, assign = _plan_balance(counts)
    nov = tpc - nprim
    cap = tpc * 128
    if ("routed2", tpc, nprim) not in _cache:
        _cache[("routed2", tpc, nprim)] = _build_routed2(tpc, nprim)
    nc = _cache[("routed2", tpc, nprim)]

    we_h = [
        np.ascontiguousarray(We[e].reshape(KT, 128, DOUT).transpose(1, 0, 2))
        .reshape(128, KT * DOUT)
        .astype(np.float16)
        for e in range(NE)
    ]
    ber_h = [
        np.ascontiguousarray(np.broadcast_to(be[e].reshape(1, DOUT), (128, DOUT)))
        for e in range(NE)
    ]
    in_maps = []
    for c in range(NCORES):
        gidx = np.zeros(cap, np.int16)
        for m, (e, s, L) in enumerate(assign[c]):
            if L:
                gidx[m * 128 : m * 128 + L] = glist[e][s : s + L]
        idx16 = np.ascontiguousarray(
            np.tile(gidx.reshape(cap // 16, 16).T, (8, 1))
        ).astype(np.int16)
        wem = np.concatenate(
            [we_h[e] for (e, s, L) in assign[c][nprim:]], axis=1
        ) if nov else np.zeros((128, 0), np.float16)
        bem = np.concatenate(
            [ber_h[e].astype(np.float16) for (e, s, L) in assign[c][nprim:]], axis=1
        ) if nov else np.zeros((128, 0), np.float16)
        in_maps.append(
            {
                "xb": xb,
                "we": we_h[c],
                "ber": ber_h[c],
                "wem": np.ascontiguousarray(wem),
                "bem": np.ascontiguousarray(bem),
                "gidx": idx16,
            }
        )

    res = run_bass_kernel_spmd(nc, in_maps, list(range(NCORES)))
    last_results = res

    out_full = np.zeros((T, DOUT), np.float32)
    # accumulate per expert (each expert's tiles partition its token list,
    # so indices are unique within one fancy-index add)
    for e in range(NE):
        ids, rows, ws = [], [], []
        for c in range(NCORES):
            yg = None
            for m, (te, s, L) in enumerate(assign[c]):
                if te == e and L:
                    if yg is None:
                        yg = np.asarray(res.results[c]["yg"])
                    ids.append(glist[e][s : s + L])
                    rows.append(yg[m * 128 : m * 128 + L])
                    ws.append(gw[e][s : s + L])
        if ids:
            ids = np.concatenate(ids)
            rows = np.concatenate(rows).astype(np.float32)
            ws = np.concatenate(ws)
            out_full[ids] += ws[:, None] * rows
    return out_full.reshape(NB, NLOC, DOUT)


NSLOT = 4  # resident weight slots per core (We-stationary design)


def _pack_slots(dtiles, nslot=NSLOT):
    """Bin-pack expert tile demands into 8 cores x nslot weight slots.

    Every core runs the same compiled program: slot s covers a fixed number
    of 128-token positions (sizes[s]); the expert that slot s serves on core
    c is data (its weights are DMA'd into SBUF slot s from a per-core param).
    Feasibility: expert e's dtiles[e] tiles are split into pieces, each piece
    living in one (core, slot) bin of capacity sizes[s].

    Returns (tpc, sizes, bins) with bins[c][s] = (expert, tile_start, ntiles).
    """
    ne = len(dtiles)
    total = sum(dtiles)

    def partitions(tpc):
        out = []
        for n0 in range((tpc + nslot - 1) // nslot, tpc - nslot + 2):
            for n1 in range(1, n0 + 1):
                for n2 in range(1, n1 + 1):
                    n3 = tpc - n0 - n1 - n2
                    if 1 <= n3 <= n2:
                        out.append((n0, n1, n2, n3))
        return out

    def min_waste_combo(d, sizes, avail):
        # exhaustive: counts per slot class <= avail, sum >= d, min waste
        best = None
        for c0 in range(avail[0] + 1):
            for c1 in range(avail[1] + 1):
                for c2 in range(avail[2] + 1):
                    for c3 in range(avail[3] + 1):
                        cap = (
                            c0 * sizes[0]
                            + c1 * sizes[1]
                            + c2 * sizes[2]
                            + c3 * sizes[3]
                        )
                        if cap < d:
                            continue
                        w = cap - d
                        nb = c0 + c1 + c2 + c3
                        if best is None or (w, nb) < best[:2]:
                            best = (w, nb, (c0, c1, c2, c3))
        return best

    for tpc in range((total + 7) // 8, (total + 7) // 8 + 24):
        for sizes in partitions(tpc):
            avail = [8] * nslot
            order = sorted(range(ne), key=lambda e: -dtiles[e])
            placements = {}
            ok = True
            for e in order:
                if dtiles[e] == 0:
                    placements[e] = (0, 0, 0, 0)
                    continue
                got = min_waste_combo(dtiles[e], sizes, avail)
                if got is None:
                    ok = False
                    break
                _, _, counts = got
                for s in range(nslot):
                    avail[s] -= counts[s]
                placements[e] = counts
            if not ok:
                continue
            # materialize bins: per slot, hand out cores 0..7 to expert pieces
            bins = [[None] * nslot for _ in range(8)]
            feas = True
            for s in range(nslot):
                core = 0
                for e in range(ne):
                    cnt = placements[e][s]
                    for _ in range(cnt):
                        if core >= 8:
                            feas = False
                            break
                        bins[core][s] = [e, 0, 0]  # start filled below
                        core += 1
                    if not feas:
                        break
            if not feas:
                continue
            # fill tile ranges expert-major: expert pieces in (slot asc, core asc)
            tile_ptr = [0] * ne
            for s in range(nslot):
                for c in range(8):
                    if bins[c][s] is None:
                        continue
                    e = bins[c][s][0]
                    take = min(sizes[s], dtiles[e] - tile_ptr[e])
                    bins[c][s] = (e, tile_ptr[e], take)
                    tile_ptr[e] += take
            if any(tile_ptr[e] != dtiles[e] for e in range(ne)):
                continue
            for c in range(8):
                for s in range(nslot):
                    if bins[c][s] is None:
                        bins[c][s] = (0, 0, 0)  # dummy slot
            return tpc, list(sizes), bins
    raise RuntimeError("slot packing failed")


def _chunks_for(cap):
    """Gather chunk sizes: 256-token warm-ups so the PE starts early (and the
    early chunk sem latency stays under the PE consumption rate), then 512s."""
    chunks = [256, 256, 256, 256] if cap > 1536 else [cap]
    rest = cap - sum(chunks)
    chunks += [512] * (rest // 512)
    if rest % 512:
        chunks.append(rest % 512)
    return chunks


def _obatches(tpc):
    """Output-DMA batch sizes: 4-position batches, tapered tail so the last
    transfers are small (the final DMA is on the program's critical path)."""
    out = []
    rem = tpc
    while rem > 5:
        out.append(4)
        rem -= 4
    while rem > 1:
        out.append(2)
        rem -= 2
    if rem:
        out.append(1)
    return out


N_WARM = 10  # dummy matmuls that hold the PE p-state ramp until real data lands
PRE = 256  # tokens prestaged by the host in gather layout (skips the SWDGE
           # round-trip for the first two positions, starting the PE ~1.5us
           # earlier)


def _build_routed3(tpc, sizes):
    """We-stationary expert-parallel program.

    Weights live in SBUF (NSLOT expert slots, per-core contents); the gathered
    tokens are the matmul moving operand ([din, token] transposed-gather
    layout), producing y^T = We^T @ x^T in PSUM [dout_blk(128), 4, 128 tok].
    Epilogue: DVE adds the per-partition bias (broadcast along tokens), ACT
    applies tanh casting to fp16, and batched DMAs stream y^T tiles out.
    """
    cap = tpc * 128
    chunks = _chunks_for(cap)
    nc = bacc.Bacc("TRN2", target_bir_lowering=False, debug=False)
    xb_d = nc.declare_dram_parameter("xb", [T, DIN], F16, isOutput=False)
    wseg_d = nc.declare_dram_parameter(
        "wseg", [128, NSLOT * KT * DOUT], F16, isOutput=False
    )
    bias_d = nc.declare_dram_parameter("bias", [128, NSLOT * KT], F32, isOutput=False)
    gidx_d = nc.declare_dram_parameter("gidx", [128, cap // 16], I16, isOutput=False)
    yg_d = nc.declare_dram_parameter("yg", [128, tpc * DOUT], F16, isOutput=True)

    # position -> slot (compile-time): consecutive blocks of sizes[s]
    slot_of = []
    for s, n in enumerate(sizes):
        slot_of += [s] * n
    # position -> (chunk index, token offset inside chunk)
    pos2chunk = []
    ci, base = 0, 0
    for p in range(tpc):
        while p * 128 >= base + chunks[ci]:
            base += chunks[ci]
            ci += 1
        pos2chunk.append((ci, p * 128 - base))
    # idx columns covered by the first two (warm-up) chunks — their table
    # loads as a tiny separate DMA so the first gather's sem chain is short
    idx_split = (chunks[0] + chunks[1]) // 16 if len(chunks) > 2 else cap // 16

    batches = _obatches(tpc)
    # position -> (batch index, offset in batch)
    pos2batch = []
    for bidx, blen in enumerate(batches):
        for j in range(blen):
            pos2batch.append((bidx, blen))
    # deferred weight-slot loads: slot s's DMA is emitted on the DVE queue
    # right after position defer_after[s]'s bias-add, so its DRAM transfer
    # stays off the DMA engines while the early gathers stream in.
    sbase_l = [0]
    for n in sizes:
        sbase_l.append(sbase_l[-1] + n)


    with TileContext(nc) as tc:
        with (
            tc.tile_pool(name="const", bufs=1) as cpool,
            tc.tile_pool(name="xg", bufs=5) as xgpool,
            tc.tile_pool(name="work", bufs=6) as wpool,
            tc.tile_pool(name="out", bufs=3) as opool,
            tc.tile_pool(name="psum", bufs=1, space="PSUM") as ppool,
        ):
            idx_sb = cpool.tile([128, cap // 16], I16)
            nc.sync.dma_start(idx_sb[:, :idx_split], gidx_d[:, :idx_split])
            we_sb = []
            for s in range(NSLOT):
                we_sb.append(cpool.tile([128, KT * DOUT], F16, name=f"we{s}"))
            nc.sync.dma_start(we_sb[0][:], wseg_d[:, 0 : KT * DOUT])
            nc.sync.dma_start(idx_sb[:, idx_split:], gidx_d[:, idx_split:])
            bias_sb = cpool.tile([128, NSLOT * KT], F32)
            nc.sync.dma_start(bias_sb[:], bias_d[:])

            # Weight slots 1..3: their DMAs must NOT race the startup gathers
            # for DMA-engine bandwidth (the scheduler hoists dep-free DMAs to
            # t=0). Chain each behind the idx-tail DMA via true data hazards:
            # anch0 reads the idx tail (RAW), anch_s reads slot s's tile (so
            # the slot DMA gets a WAR dependency on anch_s).
            anch = cpool.tile([128, 2], F32, name="anch")
            nc.vector.tensor_copy(
                anch[:, 0:1], idx_sb[:, idx_split : idx_split + 1]
            )
            HK = KT * DOUT // 2
            for s in range(1, NSLOT):
                nc.vector.tensor_tensor(
                    anch[:, 1:2], we_sb[s][:, 0:1], anch[:, 0:1],
                    mybir.AluOpType.add,
                )
                for h in range(2):
                    nc.sync.dma_start(
                        we_sb[s][:, h * HK : (h + 1) * HK],
                        wseg_d[
                            :, s * KT * DOUT + h * HK : s * KT * DOUT + (h + 1) * HK
                        ],
                    )

            # warm-up: keep the PE busy (and its p-state ramp burning) on a
            # zeroed tile until the first gather lands.
            zt = wpool.tile([128, 4, 128], F16, name="zt", tag="zt")
            nc.vector.memset(zt[:], 0)
            wps = ppool.tile([128, 4, 128], F32, name="ps7", tag="ps7")
            for _ in range(N_WARM):
                nc.tensor.matmul(wps[:], zt[:, 0, :], zt[:], start=True, stop=True)

            xgs = []
            g0 = 0
            for gi, glen in enumerate(chunks):
                xgm = xgpool.tile([128, KT, glen], F16, name="xgm", tag="xgm")
                nc.gpsimd.dma_gather(
                    out_ap=xgm[:],
                    in_ap=xb_d[:],
                    idxs_ap=idx_sb[:, g0 // 16 : (g0 + glen) // 16],
                    num_idxs=glen,
                    num_idxs_reg=glen,
                    elem_size=DIN,
                    transpose=True,
                )
                xgs.append(xgm)
                g0 += glen

            ot = None
            p0 = 0
            for p in range(tpc):
                s = slot_of[p]
                ci, off = pos2chunk[p]
                xsrc = xgs[ci]
                ps = ppool.tile([128, 4, 128], F32, name=f"ps{p % 8}", tag=f"ps{p % 8}")
                for b in range(4):
                    for k in range(KT):
                        nc.tensor.matmul(
                            ps[:, b, :],
                            we_sb[s][:, (k * 4 + b) * 128 : (k * 4 + b + 1) * 128],
                            xsrc[:, k, off : off + 128],
                            start=(k == 0),
                            stop=(k == KT - 1),
                        )
                bidx, blen = pos2batch[p]
                bi = p - p0
                if bi == 0:
                    ot = opool.tile([128, blen, 4, 128], F16, name="ot", tag="ot")
                t1 = wpool.tile([128, 4, 128], F16, tag="t1")
                bias_bc = bias_sb[:, s * 4 : (s + 1) * 4].unsqueeze(2).broadcast_to(
                    [128, 4, 128]
                )
                nc.vector.tensor_add(t1[:], ps[:], bias_bc)
                nc.scalar.activation(
                    ot[:, bi], t1[:], mybir.ActivationFunctionType.Tanh
                )
                if bi == blen - 1:
                    # final two batches issue from SP so they don't block the
                    # ACT queue's last activations (shorter tail)
                    dma_eng = nc.sync if bidx >= len(batches) - 2 else nc.scalar
                    dma_eng.dma_start(
                        yg_d[:, p0 * DOUT : (p0 + blen) * DOUT], ot[:]
                    )
                    p0 = p + 1
    nc.compile()
    return nc


def _kernel_routed3(x, type_embeddings, atom_types, Wg, We, be):
    global last_results, last_nc
    x = np.asarray(x, np.float32)
    We = np.asarray(We, np.float32)
    be = np.asarray(be, np.float32)
    _, top2_t, w_t = _routing(
        np.asarray(type_embeddings, np.float32),
        np.asarray(Wg, np.float32),
        np.asarray(atom_types),
    )
    xb = x.reshape(T, DIN).astype(np.float16)

    glist, gw = [], []
    for e in range(NE):
        sel1 = np.nonzero(top2_t[:, 0] == e)[0]
        sel2 = np.nonzero(top2_t[:, 1] == e)[0]
        toks = np.concatenate([sel1, sel2])
        ws = np.concatenate([w_t[sel1, 0], w_t[sel2, 1]])
        o = np.argsort(toks, kind="stable")
        glist.append(toks[o])
        gw.append(ws[o].astype(np.float32))
    counts = [len(g) for g in glist]
    dtiles = [(c + 127) // 128 for c in counts]

    tpc, sizes, bins = _pack_slots(dtiles)
    cap = tpc * 128
    key = ("routed3", tpc, tuple(sizes))
    if key not in _cache:
        _cache[key] = _build_routed3(tpc, sizes)
    nc = _cache[key]
    last_nc = nc

    # slot base positions
    sbase = np.concatenate([[0], np.cumsum(sizes)]).astype(int)

    # per-expert packed hosts
    we_h = [
        np.ascontiguousarray(
            We[e].reshape(KT, 128, 4, 128).transpose(1, 0, 2, 3)
        ).reshape(128, KT * DOUT).astype(np.float16)
        for e in range(NE)
    ]
    # bias per expert in [q(128), b(4)] layout
    be_h = [np.ascontiguousarray(be[e].reshape(4, 128).T) for e in range(NE)]

    in_maps = []
    for c in range(NCORES):
        gidx = np.zeros(cap, np.int16)
        wseg = np.zeros((128, NSLOT * KT * DOUT), np.float16)
        bias = np.zeros((128, NSLOT * 4), np.float32)
        for s in range(NSLOT):
            e, t0, nt = bins[c][s]
            wseg[:, s * KT * DOUT : (s + 1) * KT * DOUT] = we_h[e]
            bias[:, s * 4 : (s + 1) * 4] = be_h[e]
            if nt:
                toks = glist[e][t0 * 128 : t0 * 128 + nt * 128]
                dst0 = sbase[s] * 128
                gidx[dst0 : dst0 + len(toks)] = toks
        idx16 = np.ascontiguousarray(
            np.tile(gidx.reshape(cap // 16, 16).T, (8, 1))
        ).astype(np.int16)
        in_maps.append(
            {"xb": xb, "wseg": wseg, "bias": bias, "gidx": idx16}
        )

    res = run_bass_kernel_spmd(nc, in_maps, list(range(NCORES)))
    last_results = res

    out_full = np.zeros((T, DOUT), np.float32)
    for c in range(NCORES):
        # yg [q(128), p(tpc), b(4), r(128)]; dout = b*128+q, token = gidx[p*128+r]
        yg = np.asarray(res.results[c]["yg"]).reshape(128, tpc, 4, 128)
        for s in range(NSLOT):
            e, t0, nt = bins[c][s]
            if not nt:
                continue
            toks = glist[e][t0 * 128 : t0 * 128 + nt * 128]
            n = len(toks)
            blk = yg[:, sbase[s] : sbase[s] + nt]  # [q, nt, b, r]
            rows = blk.transpose(1, 3, 2, 0).reshape(nt * 128, DOUT)[:n]
            ws = gw[e][t0 * 128 : t0 * 128 + n]
            out_full[toks] += ws[:, None] * rows.astype(np.float32)
    return out_full.reshape(NB, NLOC, DOUT)


def kernel(x, type_embeddings, atom_types, Wg, We, be):
    global last_results
    design = os.environ.get("MOE_DESIGN", "routed3")
    if design == "routed3":
        return _kernel_routed3(x, type_embeddings, atom_types, Wg, We, be)
    if design == "routed2":
        return _kernel_routed2(x, type_embeddings, atom_types, Wg, We, be)
    if design == "routed":
        return _kernel_routed(x, type_embeddings, atom_types, Wg, We, be)
    x = np.asarray(x, np.float32)
    We = np.asarray(We, np.float32)
    be = np.asarray(be, np.float32)
    ptw, _, _ = _routing(
        np.asarray(type_embeddings, np.float32),
        np.asarray(Wg, np.float32),
        np.asarray(atom_types),
    )

    x2 = x.reshape(T, DIN)
    ber = np.ascontiguousarray(
        np.broadcast_to(be.reshape(1, NE * DOUT), (128, NE * DOUT))
    )
    # [128, NE*KT*DOUT]: we_h[p, (e*KT+k)*DOUT + d] = We[e, k*128+p, d]
    we_h = np.ascontiguousarray(
        We.reshape(NE, KT, 128, DOUT).transpose(2, 0, 1, 3)
    ).reshape(128, NE * KT * DOUT)
    in_maps = []
    for c in range(NCORES):
        x2c = x2[c * TC : (c + 1) * TC]
        # [128, KT*TC]: xt[p, k*TC + n] = x2c[n, k*128+p]
        xt = np.ascontiguousarray(
            x2c.reshape(TC, KT, 128).transpose(2, 1, 0)
        ).reshape(128, KT * TC)
        xw = np.concatenate([xt, we_h], axis=1)
        pwl = np.ascontiguousarray(
            ptw[c * TC : (c + 1) * TC].reshape(MT, 128, NE).transpose(1, 0, 2)
        ).reshape(128, MT * NE)
        in_maps.append({"xw": xw, "pwl": pwl, "ber": ber})

    if "dense" not in _cache:
        _cache["dense"] = _build_dense()
    nc = _cache["dense"]

    res = run_bass_kernel_spmd(nc, in_maps, list(range(NCORES)))
    last_results = res
    out = np.concatenate([res.results[c]["out"] for c in range(NCORES)], axis=0)
    return out.reshape(NB, NLOC, DOUT).astype(np.float32)

